# revision 23
# baseline (speedup 1.0000x reference)
"""Causal self-attention on 8 TRN2 NeuronCores (Bass/Tile, SPMD), v2.

Problem: B=4, T=2048, C=1024, H=16, D=64, fp32 in/out.

Sharding: core i = (batch b=i//2, head-half hh=i%2). Each core computes its
8 heads over ALL T=2048 positions of its batch — no K/V duplication, no
padding (exact causal prefixes), identical instruction stream on every core.
The output projection is computed against the core's own 512 O-channels,
giving a partial y[2048,1024]; the host sums the two partials per batch
(tensor-parallel reduce done host-side — there is no device collective).

All matmul operands are bf16 (fp32 PSUM accumulation): 2x less SBUF/DMA
than fp32r, enables fast weight load, measured end-to-end rel err ~5e-3
(budget 2e-2).

Per-core pipeline (one TileContext, phases interleave via the scheduler):
  V:    V_aug[m][t128, 8*(64|1)] tiles (ones col folded for softmax sums).
  K_j/Q_j: K^T/Q^T[2 heads*64d, T] per head-pair j, bias folded, Q pre-scaled.
  attn(j): per 256-wide q chunk c (prefix E=2c+2 k-tiles): S^T via row-packed
        K=64 matmuls (2 heads concurrent via tile_position), batched exp on
        ScalarE over [128,1024] PSUM, causal masks (2 constant step masks) on
        DVE for the diagonal pair only, PV accumulation into oab[65,512]
        (row 64 = softmax sums l). Normalize with DVE reciprocal + GpSimd
        partition-broadcast; head B writes partitions 64:128 directly
        (DVE partition-base shift, HW-verified).
  proj: y_partial = O^T.T @ Wproj_own + (0.5*bproj + bv_own@Wproj_own).
"""
import os
import sys
import numpy as np
import ml_dtypes

import concourse.bacc as bacc
import concourse.mybir as mybir
import concourse.tile as tile
from concourse.bass_utils import run_bass_kernel_spmd

B, T, C, H, D = 4, 2048, 1024, 16, 64
QC = 256                      # q-chunk width
NC_ = 8                       # q-chunks per core
F32 = mybir.dt.float32
BF16 = mybir.dt.bfloat16
NPBF = ml_dtypes.bfloat16

_cache = {}


def _build():
    nc = bacc.Bacc("TRN2", target_bir_lowering=False, debug=False,
                   enable_asserts=False, num_devices=8)

    def din(name, shape, dt=BF16):
        return nc.dram_tensor(name, list(shape), dt, kind="ExternalInput").ap()

    xt_d = din("xt", (C, T))                 # x[b].T, bf16
    # packed weight/const blobs (f32 pieces bitcast to 2 bf16 cols each):
    # A: wv[8x512]                                           -> 4096 cols
    # B: wk[8x512] | wq[8x512] | bk[4x2] | bq[4x2]           -> 8208 cols
    # C: wp[4x1024] | bpe[2048] | maskneg f32[2048]          -> 8192 cols
    U16 = mybir.dt.uint16
    ba_d = din("blob_a", (128, 4096), U16)
    bb_d = din("blob_b", (128, 8208), U16)
    bc_d = din("blob_c", (128, 8192), U16)
    y_d = nc.dram_tensor("y", [T, C], BF16, kind="ExternalOutput").ap()

    bypass = mybir.AluOpType.bypass
    mult = mybir.AluOpType.mult
    add = mybir.AluOpType.add
    EXP = mybir.ActivationFunctionType.Exp

    with tile.TileContext(nc) as tc:
        kp = tc.alloc_tile_pool(name="kp", bufs=1)
        # persistent SBUF tensors
        xres = [kp.tile([128, T], BF16, name=f"x{c}", tag=f"x{c}") for c in range(8)]
        blobA = kp.tile([128, 4096], mybir.dt.uint16, name="blobA", tag="blobA")
        blobB = kp.tile([128, 8208], mybir.dt.uint16, name="blobB", tag="blobB")
        blobC = kp.tile([128, 8192], mybir.dt.uint16, name="blobC", tag="blobC")
        KT = [kp.tile([128, T], BF16, name=f"kt{j}", tag=f"kt{j}") for j in range(4)]
        QT = [kp.tile([128, T], BF16, name=f"qt{j}", tag=f"qt{j}") for j in range(4)]
        OT = [kp.tile([128, T], BF16, name=f"ot{j}", tag=f"ot{j}") for j in range(4)]
        VA = [kp.tile([128, 8 * 65], BF16, name=f"va{m}", tag=f"va{m}") for m in range(16)]
        ones8 = kp.tile([128, 8], BF16, name="ones8", tag="ones8")

        wvt = [blobA[:, 512*c:512*(c+1)].bitcast(BF16) for c in range(8)]
        wkt = [blobB[:, 512*c:512*(c+1)].bitcast(BF16) for c in range(8)]
        wqt = [blobB[:, 4096+512*c:4096+512*(c+1)].bitcast(BF16) for c in range(8)]
        bks = [blobB[:, 8192+2*j:8192+2*(j+1)].bitcast(F32) for j in range(4)]
        bqs = [blobB[:, 8200+2*j:8200+2*(j+1)].bitcast(F32) for j in range(4)]
        wpt = [blobC[:, 1024*j:1024*(j+1)].bitcast(BF16) for j in range(4)]
        bpe = blobC[:, 4096:6144].bitcast(F32)
        MKN = blobC[:, 6144:8192].bitcast(F32)   # 0 / -60000 causal mask

        # ---- input DMAs: all on Sync; 11 big descriptors total ----
        nc.sync.dma_start(out=xres[0][:], in_=xt_d[0:128, :])
        nc.sync.dma_start(out=blobA[:], in_=ba_d)
        for c in range(1, 8):
            nc.sync.dma_start(out=xres[c][:], in_=xt_d[128*c:128*(c+1), :])
        nc.sync.dma_start(out=blobB[:], in_=bb_d)
        nc.sync.dma_start(out=blobC[:], in_=bc_d)

        nc.vector.memset(ones8[:], 1.0)
        for m in range(16):
            dst = VA[m][:].rearrange("p (h e) -> p h e", e=D+1)[:, :, D:D+1]
            nc.vector.tensor_copy(out=dst, in_=ones8[:].unsqueeze(2))

        with tc.tile_pool(name="ps", bufs=1, space="PSUM") as psp, \
             tc.tile_pool(name="ptp", bufs=3) as ptp, \
             tc.tile_pool(name="evp", bufs=2) as evp, \
             tc.tile_pool(name="ybp", bufs=3) as ybp:

            def v_tiles(ms):
                for m in ms:
                    s, tt = m // 4, m % 4
                    pv = psp.tile([128, 512], F32, name="acc", tag="acc", bufs=2)
                    for c in range(8):
                        nc.tensor.matmul(out=pv[:],
                                         lhsT=xres[c][:, 512*s+128*tt:512*s+128*(tt+1)],
                                         rhs=wvt[c], start=(c == 0), stop=(c == 7))
                    dst = VA[m][:].rearrange("p (h e) -> p h e", e=D+1)[:, :, 0:D]
                    nc.vector.tensor_copy(out=dst,
                                          in_=pv[:].rearrange("p (h d) -> p h d", d=D))

            def proj_tile(ti):
                for jc in range(2):
                    py = psp.tile([128, 512], F32, name="acc", tag="acc", bufs=2)
                    for j in range(4):
                        nc.tensor.matmul(out=py[:],
                                         lhsT=OT[j][:, 128*ti:128*(ti+1)],
                                         rhs=wpt[j][:, 512*jc:512*(jc+1)],
                                         start=(j == 0), stop=(j == 3))
                    ysb = ybp.tile([128, 512], BF16, name="ysb", tag="ysb")
                    nc.vector.scalar_tensor_tensor(
                        out=ysb[:], in0=py[:], scalar=0.0,
                        in1=bpe[:, 512*jc:512*(jc+1)], op0=bypass, op1=add)
                    nc.sync.dma_start(out=y_d[128*ti:128*(ti+1), 512*jc:512*(jc+1)],
                                      in_=ysb[:])

            # V slab 0 first so attention j=0 can start early; the rest of V
            # is emitted after attn(0) and fills its exp-wait PE slack.
            v_tiles(range(4))

            # ---- per head-pair: K_j, Q_j then attention ----
            for j in range(4):
                for s in range(4):
                    pk = psp.tile([128, 512], F32, name="acc", tag="acc", bufs=2)
                    for c in range(8):
                        nc.tensor.matmul(out=pk[:], lhsT=wkt[c][:, 128*j:128*(j+1)],
                                         rhs=xres[c][:, 512*s:512*(s+1)],
                                         start=(c == 0), stop=(c == 7))
                    nc.vector.tensor_scalar_add(out=KT[j][:, 512*s:512*(s+1)],
                                                in0=pk[:], scalar1=bks[j])
                    pq = psp.tile([128, 512], F32, name="acc", tag="acc", bufs=2)
                    for c in range(8):
                        nc.tensor.matmul(out=pq[:], lhsT=wqt[c][:, 128*j:128*(j+1)],
                                         rhs=xres[c][:, 512*s:512*(s+1)],
                                         start=(c == 0), stop=(c == 7))
                    nc.vector.tensor_scalar_add(out=QT[j][:, 512*s:512*(s+1)],
                                                in0=pq[:], scalar1=bqs[j])

                for cq in range(NC_):
                    E = 2 * cq + 2
                    if j == 0 and cq in (2, 4, 6):
                        # emit V tiles just before the first chunks that read
                        # them (program order = semantic order); they also
                        # fill attn(0) exp-wait slack on PE
                        v_tiles(range(2*cq, 2*cq + 4))
                    qA = QT[j][0:64, QC*cq:QC*(cq+1)]
                    qB = QT[j][64:128, QC*cq:QC*(cq+1)]
                    oab = psp.tile([65, 512], F32, name="oab", tag="oab", bufs=2)
                    for g in range(E // 2):
                        ss = psp.tile([128, 1024], F32, name="ss", tag="ss", bufs=2)
                        for u in range(2):
                            m = 2*g + u
                            nc.tensor.matmul(out=ss[:, QC*u:QC*(u+1)],
                                             lhsT=KT[j][0:64, 128*m:128*(m+1)],
                                             rhs=qA, tile_position=(0, 0),
                                             start=True, stop=True)
                            nc.tensor.matmul(out=ss[:, 512+QC*u:512+QC*(u+1)],
                                             lhsT=KT[j][64:128, 128*m:128*(m+1)],
                                             rhs=qB, tile_position=(64, 0),
                                             start=True, stop=True)
                        if g == E // 2 - 1:   # diagonal pair: m = 2c, 2c+1
                            # additive causal mask on S before exp: keeps the
                            # exp->PV chain free of DVE/GpSimd dependencies
                            nc.vector.scalar_tensor_tensor(
                                out=ss[:], in0=ss[:], scalar=0.0,
                                in1=MKN, op0=bypass, op1=add)
                        pt = ptp.tile([128, 1024], BF16, name="pt", tag="pt")
                        nc.scalar.activation(out=pt[:], in_=ss[:], func=EXP)
                        for u in range(2):
                            m = 2*g + u
                            # one has_written group per bank: only the first
                            # matmul starts it; head B's m=0 overwrite relies
                            # on the bank-wide pending-zero from head A's start
                            nc.tensor.matmul(out=oab[:, 0:QC],
                                             lhsT=VA[m][:, 65*(2*j):65*(2*j)+65],
                                             rhs=pt[:, QC*u:QC*(u+1)],
                                             start=(m == 0), stop=(m == E - 1))
                            nc.tensor.matmul(out=oab[:, QC:512],
                                             lhsT=VA[m][:, 65*(2*j+1):65*(2*j+1)+65],
                                             rhs=pt[:, 512+QC*u:512+QC*(u+1)],
                                             start=False, stop=(m == E - 1),
                                             skip_group_check=True)
                    # normalize both heads; head B lands on partitions 64:128
                    lsb = evp.tile([1, 512], F32, name="lsb", tag="lsb")
                    nc.vector.tensor_copy(out=lsb[:], in_=oab[64:65, :])
                    rsb = evp.tile([1, 512], F32, name="rsb", tag="rsb")
                    nc.vector.reciprocal_approx_fast(rsb[:], lsb[:])
                    rbb = evp.tile([64, 512], F32, name="rbb", tag="rbb")
                    nc.gpsimd.partition_broadcast(rbb[:], rsb[:])
                    nc.vector.scalar_tensor_tensor(
                        out=OT[j][0:64, QC*cq:QC*(cq+1)], in0=oab[0:64, 0:QC],
                        scalar=0.0, in1=rbb[:, 0:QC], op0=bypass, op1=mult)
                    nc.vector.scalar_tensor_tensor(
                        out=OT[j][64:128, QC*cq:QC*(cq+1)], in0=oab[0:64, QC:512],
                        scalar=0.0, in1=rbb[:, QC:512], op0=bypass, op1=mult)
                    if j == 3:   # all head-pairs done for this chunk -> project
                        proj_tile(2*cq)
                        proj_tile(2*cq + 1)
        kp.release()

    nc.compile()
    return nc


def _get_nc():
    if "nc" not in _cache:
        _cache["nc"] = _build()
    return _cache["nc"]


def _host_prep(x, Wqkv, bqkv, Wproj, bproj):
    x = np.asarray(x, dtype=np.float32)
    Wqkv = np.asarray(Wqkv, dtype=np.float32)
    bqkv = np.asarray(bqkv, dtype=np.float32)
    Wproj = np.asarray(Wproj, dtype=np.float32)
    bproj = np.asarray(bproj, dtype=np.float32)

    def f32_as_bf16(a):
        # reinterpret f32 [128,N] as its raw bits: [128,2N] bf16 columns
        return np.ascontiguousarray(a).view(NPBF)

    def chunks(w):  # [1024,512]->[128, 4096]: stack the 8 row-chunks
        return np.concatenate([w[128*c:128*(c+1), :] for c in range(8)], axis=1)

    pidx = np.arange(128)[:, None]
    fidx = np.arange(QC)[None, :]
    m0 = (pidx <= fidx)
    m1 = (128 + pidx <= fidx)
    masks = np.concatenate([m0, m1, m0, m1], axis=1).astype(NPBF)  # [128,1024]

    xts = [np.ascontiguousarray(x[b].T).astype(NPBF) for b in range(B)]
    ba_hh, bb_hh, bc_hh = [], [], []
    for hh in range(2):
        sl = slice(512*hh, 512*(hh+1))
        wq = chunks((Wqkv[:, 0:C][:, sl] * np.float32(0.125)).astype(NPBF))
        wk = chunks(Wqkv[:, C:2*C][:, sl].astype(NPBF))
        wv = chunks(Wqkv[:, 2*C:][:, sl].astype(NPBF))
        wp = np.concatenate(
            [Wproj[sl, :][128*j:128*(j+1), :].astype(NPBF) for j in range(4)], axis=1)
        bq = f32_as_bf16(bqkv[0:C][sl].reshape(4, 128).T * np.float32(0.125))  # [128,8]
        bk = f32_as_bf16(np.ascontiguousarray(bqkv[C:2*C][sl].reshape(4, 128).T))
        bv = bqkv[2*C:][sl]
        bpe = (0.5*bproj.astype(np.float64)
               + bv.astype(np.float64) @ Wproj[sl, :].astype(np.float64)).astype(np.float32)
        bpe_b = f32_as_bf16(np.broadcast_to(bpe, (128, C)).copy())  # [128,2048]
        maskneg = f32_as_bf16(np.where(masks > 0, np.float32(0), np.float32(-60000.0))
                              .astype(np.float32))  # [128,2048] u16-view
        ba_hh.append(np.ascontiguousarray(wv).view(np.uint16))
        bb_hh.append(np.ascontiguousarray(
            np.concatenate([wk, wq, bk, bq], axis=1)).view(np.uint16))
        bc_hh.append(np.ascontiguousarray(
            np.concatenate([wp, bpe_b, maskneg], axis=1)).view(np.uint16))

    in_maps = []
    for core in range(8):
        b, hh = core // 2, core % 2
        in_maps.append(dict(xt=xts[b], blob_a=ba_hh[hh], blob_b=bb_hh[hh],
                            blob_c=bc_hh[hh]))
    return in_maps


def kernel(x, Wqkv, bqkv, Wproj, bproj):
    nc = _get_nc()
    in_maps = _host_prep(x, Wqkv, bqkv, Wproj, bproj)
    trace = bool(os.environ.get("BASS_TRACE")) and "antenv.axon_hooks" in sys.modules
    res = run_bass_kernel_spmd(nc, in_maps, list(range(8)), trace=trace)
    _cache["last_exec_time_ns"] = res.exec_time_ns
    _cache["last_res"] = res
    out = np.empty((B, T, C), dtype=np.float32)
    for b in range(B):
        out[b] = np.asarray(res.results[2*b]["y"], dtype=np.float32)
        out[b] += np.asarray(res.results[2*b + 1]["y"], dtype=np.float32)
    return out


# revision 28
# speedup vs baseline: 1.0533x; 1.0533x over previous
"""Causal self-attention on 8 TRN2 NeuronCores (Bass/Tile, SPMD), v2.

Problem: B=4, T=2048, C=1024, H=16, D=64, fp32 in/out.

Sharding: core i = (batch b=i//2, head-half hh=i%2). Each core computes its
8 heads over ALL T=2048 positions of its batch — no K/V duplication, no
padding (exact causal prefixes), identical instruction stream on every core.
The output projection is computed against the core's own 512 O-channels,
giving a partial y[2048,1024]; the host sums the two partials per batch
(tensor-parallel reduce done host-side — there is no device collective).

All matmul operands are bf16 (fp32 PSUM accumulation): 2x less SBUF/DMA
than fp32r, enables fast weight load, measured end-to-end rel err ~5e-3
(budget 2e-2).

Per-core pipeline (one TileContext, phases interleave via the scheduler):
  V:    V_aug[m][t128, 8*(64|1)] tiles (ones col folded for softmax sums).
  K_j/Q_j: K^T/Q^T[2 heads*64d, T] per head-pair j, bias folded, Q pre-scaled.
  attn(j): per 256-wide q chunk c (prefix E=2c+2 k-tiles): S^T via row-packed
        K=64 matmuls (2 heads concurrent via tile_position), batched exp on
        ScalarE over [128,1024] PSUM, causal masks (2 constant step masks) on
        DVE for the diagonal pair only, PV accumulation into oab[65,512]
        (row 64 = softmax sums l). Normalize with DVE reciprocal + GpSimd
        partition-broadcast; head B writes partitions 64:128 directly
        (DVE partition-base shift, HW-verified).
  proj: y_partial = O^T.T @ Wproj_own + (0.5*bproj + bv_own@Wproj_own).
"""
import os
import sys
import numpy as np
import ml_dtypes

import concourse.bacc as bacc
import concourse.mybir as mybir
import concourse.tile as tile
from concourse.bass_utils import run_bass_kernel_spmd

B, T, C, H, D = 4, 2048, 1024, 16, 64
QC = 256                      # q-chunk width
NC_ = 8                       # q-chunks per core
F32 = mybir.dt.float32
BF16 = mybir.dt.bfloat16
NPBF = ml_dtypes.bfloat16

_cache = {}


def _build():
    nc = bacc.Bacc("TRN2", target_bir_lowering=False, debug=False,
                   enable_asserts=False, num_devices=8)

    def din(name, shape, dt=BF16):
        return nc.dram_tensor(name, list(shape), dt, kind="ExternalInput").ap()

    xt_d = din("xt", (C, T))                 # x[b].T, bf16
    # packed weight/const blobs (f32 pieces bitcast to 2 bf16 cols each):
    # A: wv[8x512]                                           -> 4096 cols
    # B: wk[8x512] | wq[8x512] | bk[4x2] | bq[4x2]           -> 8208 cols
    # C: wp[4x1024] | bpe[2048] | masks bf16[1024]           -> 7168 cols
    U16 = mybir.dt.uint16
    ba_d = din("blob_a", (128, 4096), U16)
    bb_d = din("blob_b", (128, 8208), U16)
    bc_d = din("blob_c", (128, 7168), U16)
    y_d = nc.dram_tensor("y", [T, C], BF16, kind="ExternalOutput").ap()

    bypass = mybir.AluOpType.bypass
    mult = mybir.AluOpType.mult
    add = mybir.AluOpType.add
    EXP = mybir.ActivationFunctionType.Exp

    with tile.TileContext(nc) as tc:
        kp = tc.alloc_tile_pool(name="kp", bufs=1)
        # persistent SBUF tensors
        xres = [kp.tile([128, T], BF16, name=f"x{c}", tag=f"x{c}") for c in range(8)]
        blobA = kp.tile([128, 4096], mybir.dt.uint16, name="blobA", tag="blobA")
        blobB = kp.tile([128, 8208], mybir.dt.uint16, name="blobB", tag="blobB")
        blobC = kp.tile([128, 7168], mybir.dt.uint16, name="blobC", tag="blobC")
        KT = [kp.tile([128, T], BF16, name=f"kt{j}", tag=f"kt{j}") for j in range(4)]
        QT = [kp.tile([128, T], BF16, name=f"qt{j}", tag=f"qt{j}") for j in range(4)]
        OT = [kp.tile([128, T], BF16, name=f"ot{j}", tag=f"ot{j}") for j in range(4)]
        VA = [kp.tile([128, 8 * 65], BF16, name=f"va{m}", tag=f"va{m}") for m in range(16)]
        ones8 = kp.tile([128, 8], BF16, name="ones8", tag="ones8")

        wvt = [blobA[:, 512*c:512*(c+1)].bitcast(BF16) for c in range(8)]
        wkt = [blobB[:, 512*c:512*(c+1)].bitcast(BF16) for c in range(8)]
        wqt = [blobB[:, 4096+512*c:4096+512*(c+1)].bitcast(BF16) for c in range(8)]
        bks = [blobB[:, 8192+2*j:8192+2*(j+1)].bitcast(F32) for j in range(4)]
        bqs = [blobB[:, 8200+2*j:8200+2*(j+1)].bitcast(F32) for j in range(4)]
        wpt = [blobC[:, 1024*j:1024*(j+1)].bitcast(BF16) for j in range(4)]
        bpe = blobC[:, 4096:6144].bitcast(F32)
        MK = blobC[:, 6144:7168].bitcast(BF16)   # 1/0 causal mask, fused layout

        # ---- input DMAs: all on Sync; 11 big descriptors total ----
        nc.sync.dma_start(out=xres[0][:], in_=xt_d[0:128, :])
        nc.sync.dma_start(out=blobA[:], in_=ba_d)
        for c in range(1, 8):
            nc.sync.dma_start(out=xres[c][:], in_=xt_d[128*c:128*(c+1), :])
        nc.sync.dma_start(out=blobB[:], in_=bb_d)
        nc.sync.dma_start(out=blobC[:], in_=bc_d)

        nc.vector.memset(ones8[:], 1.0)
        for m in range(16):
            dst = VA[m][:].rearrange("p (h e) -> p h e", e=D+1)[:, :, D:D+1]
            nc.vector.tensor_copy(out=dst, in_=ones8[:].unsqueeze(2))

        with tc.tile_pool(name="ps", bufs=1, space="PSUM") as psp, \
             tc.tile_pool(name="ptp", bufs=3) as ptp, \
             tc.tile_pool(name="evp", bufs=2) as evp, \
             tc.tile_pool(name="ybp", bufs=3) as ybp:

            def v_tiles(ms):
                for m in ms:
                    s, tt = m // 4, m % 4
                    pv = psp.tile([128, 512], F32, name="acc", tag="acc", bufs=2)
                    for c in range(8):
                        nc.tensor.matmul(out=pv[:],
                                         lhsT=xres[c][:, 512*s+128*tt:512*s+128*(tt+1)],
                                         rhs=wvt[c], start=(c == 0), stop=(c == 7))
                    dst = VA[m][:].rearrange("p (h e) -> p h e", e=D+1)[:, :, 0:D]
                    nc.vector.tensor_copy(out=dst,
                                          in_=pv[:].rearrange("p (h d) -> p h d", d=D))

            def proj_tile(ti):
                for jc in range(2):
                    py = psp.tile([128, 512], F32, name="acc", tag="acc", bufs=2)
                    for j in range(4):
                        nc.tensor.matmul(out=py[:],
                                         lhsT=OT[j][:, 128*ti:128*(ti+1)],
                                         rhs=wpt[j][:, 512*jc:512*(jc+1)],
                                         start=(j == 0), stop=(j == 3))
                    ysb = ybp.tile([128, 512], BF16, name="ysb", tag="ysb")
                    nc.vector.scalar_tensor_tensor(
                        out=ysb[:], in0=py[:], scalar=0.0,
                        in1=bpe[:, 512*jc:512*(jc+1)], op0=bypass, op1=add)
                    nc.sync.dma_start(out=y_d[128*ti:128*(ti+1), 512*jc:512*(jc+1)],
                                      in_=ysb[:])

            def normalize(j, cq, oab):
                # both heads; head B lands on partitions 64:128 (DVE
                # partition-base shift, HW-verified). Deferred one chunk so
                # the GpSimd broadcast latency never sits inside the next
                # diagonal mask -> PV dependency window on the DVE queue.
                lsb = evp.tile([1, 512], F32, name="lsb", tag="lsb")
                nc.vector.tensor_copy(out=lsb[:], in_=oab[64:65, :])
                rsb = evp.tile([1, 512], F32, name="rsb", tag="rsb")
                nc.vector.reciprocal_approx_fast(rsb[:], lsb[:])
                rbb = evp.tile([64, 512], F32, name="rbb", tag="rbb")
                nc.gpsimd.partition_broadcast(rbb[:], rsb[:])
                nc.vector.scalar_tensor_tensor(
                    out=OT[j][0:64, QC*cq:QC*(cq+1)], in0=oab[0:64, 0:QC],
                    scalar=0.0, in1=rbb[:, 0:QC], op0=bypass, op1=mult)
                nc.vector.scalar_tensor_tensor(
                    out=OT[j][64:128, QC*cq:QC*(cq+1)], in0=oab[0:64, QC:512],
                    scalar=0.0, in1=rbb[:, QC:512], op0=bypass, op1=mult)
                if j == 3:   # all head-pairs done for this chunk -> project
                    proj_tile(2*cq)
                    proj_tile(2*cq + 1)

            # V slab 0 first so attention j=0 can start early; the rest of V
            # is emitted after attn(0) and fills its exp-wait PE slack.
            v_tiles(range(4))
            pending = None

            # ---- per head-pair: K_j, Q_j then attention ----
            for j in range(4):
                for s in range(4):
                    pk = psp.tile([128, 512], F32, name="acc", tag="acc", bufs=2)
                    for c in range(8):
                        nc.tensor.matmul(out=pk[:], lhsT=wkt[c][:, 128*j:128*(j+1)],
                                         rhs=xres[c][:, 512*s:512*(s+1)],
                                         start=(c == 0), stop=(c == 7))
                    nc.vector.tensor_scalar_add(out=KT[j][:, 512*s:512*(s+1)],
                                                in0=pk[:], scalar1=bks[j])
                    pq = psp.tile([128, 512], F32, name="acc", tag="acc", bufs=2)
                    for c in range(8):
                        nc.tensor.matmul(out=pq[:], lhsT=wqt[c][:, 128*j:128*(j+1)],
                                         rhs=xres[c][:, 512*s:512*(s+1)],
                                         start=(c == 0), stop=(c == 7))
                    nc.vector.tensor_scalar_add(out=QT[j][:, 512*s:512*(s+1)],
                                                in0=pq[:], scalar1=bqs[j])

                for cq in range(NC_):
                    E = 2 * cq + 2
                    if j == 0 and cq in (2, 4, 6):
                        # emit V tiles just before the first chunks that read
                        # them (program order = semantic order); they also
                        # fill attn(0) exp-wait slack on PE
                        v_tiles(range(2*cq, 2*cq + 4))
                    qA = QT[j][0:64, QC*cq:QC*(cq+1)]
                    qB = QT[j][64:128, QC*cq:QC*(cq+1)]
                    oab = psp.tile([65, 512], F32, name="oab", tag="oab", bufs=2)
                    for g in range(E // 2):
                        ss = psp.tile([128, 1024], F32, name="ss", tag="ss", bufs=2)
                        for u in range(2):
                            m = 2*g + u
                            nc.tensor.matmul(out=ss[:, QC*u:QC*(u+1)],
                                             lhsT=KT[j][0:64, 128*m:128*(m+1)],
                                             rhs=qA, tile_position=(0, 0),
                                             start=True, stop=True)
                            nc.tensor.matmul(out=ss[:, 512+QC*u:512+QC*(u+1)],
                                             lhsT=KT[j][64:128, 128*m:128*(m+1)],
                                             rhs=qB, tile_position=(64, 0),
                                             start=True, stop=True)
                        pt = ptp.tile([128, 1024], BF16, name="pt", tag="pt")
                        nc.scalar.activation(out=pt[:], in_=ss[:], func=EXP)
                        if g == E // 2 - 1:   # diagonal pair: m = 2c, 2c+1
                            nc.vector.scalar_tensor_tensor(
                                out=pt[:], in0=pt[:], scalar=0.0,
                                in1=MK, op0=bypass, op1=mult)
                        for u in range(2):
                            m = 2*g + u
                            # one has_written group per bank: only the first
                            # matmul starts it; head B's m=0 overwrite relies
                            # on the bank-wide pending-zero from head A's start
                            nc.tensor.matmul(out=oab[:, 0:QC],
                                             lhsT=VA[m][:, 65*(2*j):65*(2*j)+65],
                                             rhs=pt[:, QC*u:QC*(u+1)],
                                             start=(m == 0), stop=(m == E - 1))
                            nc.tensor.matmul(out=oab[:, QC:512],
                                             lhsT=VA[m][:, 65*(2*j+1):65*(2*j+1)+65],
                                             rhs=pt[:, 512+QC*u:512+QC*(u+1)],
                                             start=False, stop=(m == E - 1),
                                             skip_group_check=True)
                    if pending is not None:
                        normalize(*pending)
                    pending = (j, cq, oab)
            normalize(*pending)
        kp.release()

    nc.compile()
    return nc


def _get_nc():
    if "nc" not in _cache:
        _cache["nc"] = _build()
    return _cache["nc"]


def _host_prep(x, Wqkv, bqkv, Wproj, bproj):
    x = np.asarray(x, dtype=np.float32)
    Wqkv = np.asarray(Wqkv, dtype=np.float32)
    bqkv = np.asarray(bqkv, dtype=np.float32)
    Wproj = np.asarray(Wproj, dtype=np.float32)
    bproj = np.asarray(bproj, dtype=np.float32)

    def f32_as_bf16(a):
        # reinterpret f32 [128,N] as its raw bits: [128,2N] bf16 columns
        return np.ascontiguousarray(a).view(NPBF)

    def chunks(w):  # [1024,512]->[128, 4096]: stack the 8 row-chunks
        return np.concatenate([w[128*c:128*(c+1), :] for c in range(8)], axis=1)

    pidx = np.arange(128)[:, None]
    fidx = np.arange(QC)[None, :]
    m0 = (pidx <= fidx)
    m1 = (128 + pidx <= fidx)
    masks = np.concatenate([m0, m1, m0, m1], axis=1).astype(NPBF)  # [128,1024]

    xts = [np.ascontiguousarray(x[b].T).astype(NPBF) for b in range(B)]
    ba_hh, bb_hh, bc_hh = [], [], []
    for hh in range(2):
        sl = slice(512*hh, 512*(hh+1))
        wq = chunks((Wqkv[:, 0:C][:, sl] * np.float32(0.125)).astype(NPBF))
        wk = chunks(Wqkv[:, C:2*C][:, sl].astype(NPBF))
        wv = chunks(Wqkv[:, 2*C:][:, sl].astype(NPBF))
        wp = np.concatenate(
            [Wproj[sl, :][128*j:128*(j+1), :].astype(NPBF) for j in range(4)], axis=1)
        bq = f32_as_bf16(bqkv[0:C][sl].reshape(4, 128).T * np.float32(0.125))  # [128,8]
        bk = f32_as_bf16(np.ascontiguousarray(bqkv[C:2*C][sl].reshape(4, 128).T))
        bv = bqkv[2*C:][sl]
        bpe = (0.5*bproj.astype(np.float64)
               + bv.astype(np.float64) @ Wproj[sl, :].astype(np.float64)).astype(np.float32)
        bpe_b = f32_as_bf16(np.broadcast_to(bpe, (128, C)).copy())  # [128,2048]
        ba_hh.append(np.ascontiguousarray(wv).view(np.uint16))
        bb_hh.append(np.ascontiguousarray(
            np.concatenate([wk, wq, bk, bq], axis=1)).view(np.uint16))
        bc_hh.append(np.ascontiguousarray(
            np.concatenate([wp.view(np.uint16), bpe_b.view(np.uint16),
                            masks.view(np.uint16)], axis=1)))

    in_maps = []
    for core in range(8):
        b, hh = core // 2, core % 2
        in_maps.append(dict(xt=xts[b], blob_a=ba_hh[hh], blob_b=bb_hh[hh],
                            blob_c=bc_hh[hh]))
    return in_maps


def kernel(x, Wqkv, bqkv, Wproj, bproj):
    nc = _get_nc()
    in_maps = _host_prep(x, Wqkv, bqkv, Wproj, bproj)
    trace = bool(os.environ.get("BASS_TRACE")) and "antenv.axon_hooks" in sys.modules
    res = run_bass_kernel_spmd(nc, in_maps, list(range(8)), trace=trace)
    _cache["last_exec_time_ns"] = res.exec_time_ns
    _cache["last_res"] = res
    out = np.empty((B, T, C), dtype=np.float32)
    for b in range(B):
        out[b] = np.asarray(res.results[2*b]["y"], dtype=np.float32)
        out[b] += np.asarray(res.results[2*b + 1]["y"], dtype=np.float32)
    return out


# revision 30
# speedup vs baseline: 1.0704x; 1.0163x over previous
"""Causal self-attention on 8 TRN2 NeuronCores (Bass/Tile, SPMD), v2.

Problem: B=4, T=2048, C=1024, H=16, D=64, fp32 in/out.

Sharding: core i = (batch b=i//2, head-half hh=i%2). Each core computes its
8 heads over ALL T=2048 positions of its batch — no K/V duplication, no
padding (exact causal prefixes), identical instruction stream on every core.
The output projection is computed against the core's own 512 O-channels,
giving a partial y[2048,1024]; the host sums the two partials per batch
(tensor-parallel reduce done host-side — there is no device collective).

All matmul operands are bf16 (fp32 PSUM accumulation): 2x less SBUF/DMA
than fp32r, enables fast weight load, measured end-to-end rel err ~5e-3
(budget 2e-2).

Per-core pipeline (one TileContext, phases interleave via the scheduler):
  V:    V_aug[m][t128, 8*(64|1)] tiles (ones col folded for softmax sums).
  K_j/Q_j: K^T/Q^T[2 heads*64d, T] per head-pair j, bias folded, Q pre-scaled.
  attn(j): per 256-wide q chunk c (prefix E=2c+2 k-tiles): S^T via row-packed
        K=64 matmuls (2 heads concurrent via tile_position), batched exp on
        ScalarE over [128,1024] PSUM, causal masks (2 constant step masks) on
        DVE for the diagonal pair only, PV accumulation into oab[65,512]
        (row 64 = softmax sums l). Normalize with DVE reciprocal + GpSimd
        partition-broadcast; head B writes partitions 64:128 directly
        (DVE partition-base shift, HW-verified).
  proj: y_partial = O^T.T @ Wproj_own + (0.5*bproj + bv_own@Wproj_own).
"""
import os
import sys
import numpy as np
import ml_dtypes

import concourse.bacc as bacc
import concourse.mybir as mybir
import concourse.tile as tile
from concourse.bass_utils import run_bass_kernel_spmd

B, T, C, H, D = 4, 2048, 1024, 16, 64
QC = 256                      # q-chunk width
NC_ = 8                       # q-chunks per core
F32 = mybir.dt.float32
BF16 = mybir.dt.bfloat16
NPBF = ml_dtypes.bfloat16

_cache = {}


def _build():
    nc = bacc.Bacc("TRN2", target_bir_lowering=False, debug=False,
                   enable_asserts=False, num_devices=8)

    def din(name, shape, dt=BF16):
        return nc.dram_tensor(name, list(shape), dt, kind="ExternalInput").ap()

    xt_d = din("xt", (C, T))                 # x[b].T, bf16
    # packed weight/const blobs (f32 pieces bitcast to 2 bf16 cols each):
    # A: wv[8x512]                                           -> 4096 cols
    # B: wk[8x512] | wq[8x512] | bk[4x2] | bq[4x2]           -> 8208 cols
    # C: wp[4x1024] | bpe[2048] | masks bf16[768]            -> 6912 cols
    U16 = mybir.dt.uint16
    ba_d = din("blob_a", (128, 4096), U16)
    bb_d = din("blob_b", (128, 8208), U16)
    bc_d = din("blob_c", (128, 6912), U16)
    y_d = nc.dram_tensor("y", [T, C], BF16, kind="ExternalOutput").ap()

    bypass = mybir.AluOpType.bypass
    mult = mybir.AluOpType.mult
    add = mybir.AluOpType.add
    EXP = mybir.ActivationFunctionType.Exp

    with tile.TileContext(nc) as tc:
        kp = tc.alloc_tile_pool(name="kp", bufs=1)
        # persistent SBUF tensors
        xres = [kp.tile([128, T], BF16, name=f"x{c}", tag=f"x{c}") for c in range(8)]
        blobA = kp.tile([128, 4096], mybir.dt.uint16, name="blobA", tag="blobA")
        blobB = kp.tile([128, 8208], mybir.dt.uint16, name="blobB", tag="blobB")
        blobC = kp.tile([128, 6912], mybir.dt.uint16, name="blobC", tag="blobC")
        KT = [kp.tile([128, T], BF16, name=f"kt{j}", tag=f"kt{j}") for j in range(4)]
        QT = [kp.tile([128, T], BF16, name=f"qt{j}", tag=f"qt{j}") for j in range(4)]
        OT = [kp.tile([128, T], BF16, name=f"ot{j}", tag=f"ot{j}") for j in range(4)]
        VA = [kp.tile([128, 8 * 65], BF16, name=f"va{m}", tag=f"va{m}") for m in range(16)]
        ones8 = kp.tile([128, 8], BF16, name="ones8", tag="ones8")

        wvt = [blobA[:, 512*c:512*(c+1)].bitcast(BF16) for c in range(8)]
        wkt = [blobB[:, 512*c:512*(c+1)].bitcast(BF16) for c in range(8)]
        wqt = [blobB[:, 4096+512*c:4096+512*(c+1)].bitcast(BF16) for c in range(8)]
        bks = [blobB[:, 8192+2*j:8192+2*(j+1)].bitcast(F32) for j in range(4)]
        bqs = [blobB[:, 8200+2*j:8200+2*(j+1)].bitcast(F32) for j in range(4)]
        wpt = [blobC[:, 1024*j:1024*(j+1)].bitcast(BF16) for j in range(4)]
        bpe = blobC[:, 4096:6144].bitcast(F32)
        MK = blobC[:, 6144:6912].bitcast(BF16)   # 1/0 causal mask, fused layout

        # ---- input DMAs: all on Sync; 11 big descriptors total ----
        nc.sync.dma_start(out=xres[0][:], in_=xt_d[0:128, :])
        nc.sync.dma_start(out=blobA[:], in_=ba_d)
        for c in range(1, 8):
            nc.sync.dma_start(out=xres[c][:], in_=xt_d[128*c:128*(c+1), :])
        nc.sync.dma_start(out=blobB[:], in_=bb_d)
        nc.sync.dma_start(out=blobC[:], in_=bc_d)

        nc.vector.memset(ones8[:], 1.0)
        for m in range(16):
            dst = VA[m][:].rearrange("p (h e) -> p h e", e=D+1)[:, :, D:D+1]
            nc.vector.tensor_copy(out=dst, in_=ones8[:].unsqueeze(2))

        with tc.tile_pool(name="ps", bufs=1, space="PSUM") as psp, \
             tc.tile_pool(name="ptp", bufs=3) as ptp, \
             tc.tile_pool(name="evp", bufs=2) as evp, \
             tc.tile_pool(name="ybp", bufs=3) as ybp:

            def v_tiles(ms):
                for m in ms:
                    s, tt = m // 4, m % 4
                    pv = psp.tile([128, 512], F32, name="acc", tag="acc", bufs=2)
                    for c in range(8):
                        nc.tensor.matmul(out=pv[:],
                                         lhsT=xres[c][:, 512*s+128*tt:512*s+128*(tt+1)],
                                         rhs=wvt[c], start=(c == 0), stop=(c == 7))
                    dst = VA[m][:].rearrange("p (h e) -> p h e", e=D+1)[:, :, 0:D]
                    nc.vector.tensor_copy(out=dst,
                                          in_=pv[:].rearrange("p (h d) -> p h d", d=D))

            def proj_tile(ti):
                for jc in range(2):
                    py = psp.tile([128, 512], F32, name="acc", tag="acc", bufs=2)
                    for j in range(4):
                        nc.tensor.matmul(out=py[:],
                                         lhsT=OT[j][:, 128*ti:128*(ti+1)],
                                         rhs=wpt[j][:, 512*jc:512*(jc+1)],
                                         start=(j == 0), stop=(j == 3))
                    ysb = ybp.tile([128, 512], BF16, name="ysb", tag="ysb")
                    nc.vector.scalar_tensor_tensor(
                        out=ysb[:], in0=py[:], scalar=0.0,
                        in1=bpe[:, 512*jc:512*(jc+1)], op0=bypass, op1=add)
                    nc.sync.dma_start(out=y_d[128*ti:128*(ti+1), 512*jc:512*(jc+1)],
                                      in_=ysb[:])

            def normalize(j, cq, oab):
                # both heads; head B lands on partitions 64:128 (DVE
                # partition-base shift, HW-verified). Deferred one chunk so
                # the GpSimd broadcast latency never sits inside the next
                # diagonal mask -> PV dependency window on the DVE queue.
                lsb = evp.tile([1, 512], F32, name="lsb", tag="lsb")
                nc.vector.tensor_copy(out=lsb[:], in_=oab[64:65, :])
                rsb = evp.tile([1, 512], F32, name="rsb", tag="rsb")
                nc.vector.reciprocal_approx_fast(rsb[:], lsb[:])
                rbb = evp.tile([64, 512], F32, name="rbb", tag="rbb")
                nc.gpsimd.partition_broadcast(rbb[:], rsb[:])
                nc.vector.scalar_tensor_tensor(
                    out=OT[j][0:64, QC*cq:QC*(cq+1)], in0=oab[0:64, 0:QC],
                    scalar=0.0, in1=rbb[:, 0:QC], op0=bypass, op1=mult)
                nc.vector.scalar_tensor_tensor(
                    out=OT[j][64:128, QC*cq:QC*(cq+1)], in0=oab[0:64, QC:512],
                    scalar=0.0, in1=rbb[:, QC:512], op0=bypass, op1=mult)
                if j == 3:   # all head-pairs done for this chunk -> project
                    proj_tile(2*cq)
                    proj_tile(2*cq + 1)

            # V slab 0 first so attention j=0 can start early; the rest of V
            # is emitted after attn(0) and fills its exp-wait PE slack.
            v_tiles(range(4))
            pending = None

            # ---- per head-pair: K_j, Q_j then attention ----
            for j in range(4):
                for s in range(4):
                    pk = psp.tile([128, 512], F32, name="acc", tag="acc", bufs=2)
                    for c in range(8):
                        nc.tensor.matmul(out=pk[:], lhsT=wkt[c][:, 128*j:128*(j+1)],
                                         rhs=xres[c][:, 512*s:512*(s+1)],
                                         start=(c == 0), stop=(c == 7))
                    nc.vector.tensor_scalar_add(out=KT[j][:, 512*s:512*(s+1)],
                                                in0=pk[:], scalar1=bks[j])
                    pq = psp.tile([128, 512], F32, name="acc", tag="acc", bufs=2)
                    for c in range(8):
                        nc.tensor.matmul(out=pq[:], lhsT=wqt[c][:, 128*j:128*(j+1)],
                                         rhs=xres[c][:, 512*s:512*(s+1)],
                                         start=(c == 0), stop=(c == 7))
                    nc.vector.tensor_scalar_add(out=QT[j][:, 512*s:512*(s+1)],
                                                in0=pq[:], scalar1=bqs[j])

                for cq in range(NC_):
                    E = 2 * cq + 2
                    if j == 0 and cq in (2, 4, 6):
                        # emit V tiles just before the first chunks that read
                        # them (program order = semantic order); they also
                        # fill attn(0) exp-wait slack on PE
                        v_tiles(range(2*cq, 2*cq + 4))
                    qA = QT[j][0:64, QC*cq:QC*(cq+1)]
                    qB = QT[j][64:128, QC*cq:QC*(cq+1)]
                    vA = lambda m: VA[m][:, 65*(2*j):65*(2*j)+65]
                    vB = lambda m: VA[m][:, 65*(2*j+1):65*(2*j+1)+65]
                    oab = psp.tile([65, 512], F32, name="oab", tag="oab", bufs=2)
                    for g in range(E // 2 - 1):   # full (unmasked) pairs
                        ss = psp.tile([128, 1024], F32, name="ss", tag="ss", bufs=2)
                        for u in range(2):
                            m = 2*g + u
                            nc.tensor.matmul(out=ss[:, QC*u:QC*(u+1)],
                                             lhsT=KT[j][0:64, 128*m:128*(m+1)],
                                             rhs=qA, tile_position=(0, 0),
                                             start=True, stop=True)
                            nc.tensor.matmul(out=ss[:, 512+QC*u:512+QC*(u+1)],
                                             lhsT=KT[j][64:128, 128*m:128*(m+1)],
                                             rhs=qB, tile_position=(64, 0),
                                             start=True, stop=True)
                        pt = ptp.tile([128, 1024], BF16, name="pt", tag="pt")
                        nc.scalar.activation(out=pt[:], in_=ss[:], func=EXP)
                        for u in range(2):
                            m = 2*g + u
                            # one has_written group per bank: only the first
                            # matmul starts it; head B's m=0 overwrite relies
                            # on the bank-wide pending-zero from head A's start
                            nc.tensor.matmul(out=oab[:, 0:QC], lhsT=vA(m),
                                             rhs=pt[:, QC*u:QC*(u+1)],
                                             start=(m == 0), stop=False,
                                             skip_group_check=True)
                            nc.tensor.matmul(out=oab[:, QC:512], lhsT=vB(m),
                                             rhs=pt[:, 512+QC*u:512+QC*(u+1)],
                                             start=False, stop=False,
                                             skip_group_check=True)
                    # diagonal pair (m = E-2 full+masked, m = E-1 upper half)
                    # ss layout: [m0A(256) | m1A(128) | m0B(256) | m1B(128)]
                    md = E - 2
                    ss = psp.tile([128, 1024], F32, name="ss", tag="ss", bufs=2)
                    nc.tensor.matmul(out=ss[:, 0:QC],
                                     lhsT=KT[j][0:64, 128*md:128*(md+1)],
                                     rhs=qA, tile_position=(0, 0),
                                     start=True, stop=True)
                    nc.tensor.matmul(out=ss[:, QC:384],
                                     lhsT=KT[j][0:64, 128*(md+1):128*(md+2)],
                                     rhs=qA[:, 128:QC], tile_position=(0, 0),
                                     start=True, stop=True)
                    nc.tensor.matmul(out=ss[:, 384:640],
                                     lhsT=KT[j][64:128, 128*md:128*(md+1)],
                                     rhs=qB, tile_position=(64, 0),
                                     start=True, stop=True)
                    nc.tensor.matmul(out=ss[:, 640:768],
                                     lhsT=KT[j][64:128, 128*(md+1):128*(md+2)],
                                     rhs=qB[:, 128:QC], tile_position=(64, 0),
                                     start=True, stop=True)
                    pt = ptp.tile([128, 1024], BF16, name="pt", tag="pt")
                    nc.scalar.activation(out=pt[:, 0:768], in_=ss[:, 0:768], func=EXP)
                    nc.vector.scalar_tensor_tensor(
                        out=pt[:, 0:768], in0=pt[:, 0:768], scalar=0.0,
                        in1=MK, op0=bypass, op1=mult)
                    nc.tensor.matmul(out=oab[:, 0:QC], lhsT=vA(md),
                                     rhs=pt[:, 0:QC],
                                     start=(md == 0), stop=False,
                                     skip_group_check=True)
                    nc.tensor.matmul(out=oab[:, 128:QC], lhsT=vA(md+1),
                                     rhs=pt[:, QC:384],
                                     start=False, stop=True, skip_group_check=True)
                    nc.tensor.matmul(out=oab[:, QC:512], lhsT=vB(md),
                                     rhs=pt[:, 384:640],
                                     start=False, stop=False, skip_group_check=True)
                    nc.tensor.matmul(out=oab[:, QC+128:512], lhsT=vB(md+1),
                                     rhs=pt[:, 640:768],
                                     start=False, stop=True, skip_group_check=True)
                    if pending is not None:
                        normalize(*pending)
                    pending = (j, cq, oab)
            normalize(*pending)
        kp.release()

    nc.compile()
    return nc


def _get_nc():
    if "nc" not in _cache:
        _cache["nc"] = _build()
    return _cache["nc"]


def _host_prep(x, Wqkv, bqkv, Wproj, bproj):
    x = np.asarray(x, dtype=np.float32)
    Wqkv = np.asarray(Wqkv, dtype=np.float32)
    bqkv = np.asarray(bqkv, dtype=np.float32)
    Wproj = np.asarray(Wproj, dtype=np.float32)
    bproj = np.asarray(bproj, dtype=np.float32)

    def f32_as_bf16(a):
        # reinterpret f32 [128,N] as its raw bits: [128,2N] bf16 columns
        return np.ascontiguousarray(a).view(NPBF)

    def chunks(w):  # [1024,512]->[128, 4096]: stack the 8 row-chunks
        return np.concatenate([w[128*c:128*(c+1), :] for c in range(8)], axis=1)

    pidx = np.arange(128)[:, None]
    fidx = np.arange(QC)[None, :]
    m0 = (pidx <= fidx)
    m0h = m0[:, 0:128]
    masks = np.concatenate([m0, m0h, m0, m0h], axis=1).astype(NPBF)  # [128,768]

    xts = [np.ascontiguousarray(x[b].T).astype(NPBF) for b in range(B)]
    ba_hh, bb_hh, bc_hh = [], [], []
    for hh in range(2):
        sl = slice(512*hh, 512*(hh+1))
        wq = chunks((Wqkv[:, 0:C][:, sl] * np.float32(0.125)).astype(NPBF))
        wk = chunks(Wqkv[:, C:2*C][:, sl].astype(NPBF))
        wv = chunks(Wqkv[:, 2*C:][:, sl].astype(NPBF))
        wp = np.concatenate(
            [Wproj[sl, :][128*j:128*(j+1), :].astype(NPBF) for j in range(4)], axis=1)
        bq = f32_as_bf16(bqkv[0:C][sl].reshape(4, 128).T * np.float32(0.125))  # [128,8]
        bk = f32_as_bf16(np.ascontiguousarray(bqkv[C:2*C][sl].reshape(4, 128).T))
        bv = bqkv[2*C:][sl]
        bpe = (0.5*bproj.astype(np.float64)
               + bv.astype(np.float64) @ Wproj[sl, :].astype(np.float64)).astype(np.float32)
        bpe_b = f32_as_bf16(np.broadcast_to(bpe, (128, C)).copy())  # [128,2048]
        ba_hh.append(np.ascontiguousarray(wv).view(np.uint16))
        bb_hh.append(np.ascontiguousarray(
            np.concatenate([wk, wq, bk, bq], axis=1)).view(np.uint16))
        bc_hh.append(np.ascontiguousarray(
            np.concatenate([wp.view(np.uint16), bpe_b.view(np.uint16),
                            masks.view(np.uint16)], axis=1)))

    in_maps = []
    for core in range(8):
        b, hh = core // 2, core % 2
        in_maps.append(dict(xt=xts[b], blob_a=ba_hh[hh], blob_b=bb_hh[hh],
                            blob_c=bc_hh[hh]))
    return in_maps


def kernel(x, Wqkv, bqkv, Wproj, bproj):
    nc = _get_nc()
    in_maps = _host_prep(x, Wqkv, bqkv, Wproj, bproj)
    trace = bool(os.environ.get("BASS_TRACE")) and "antenv.axon_hooks" in sys.modules
    res = run_bass_kernel_spmd(nc, in_maps, list(range(8)), trace=trace)
    _cache["last_exec_time_ns"] = res.exec_time_ns
    _cache["last_res"] = res
    out = np.empty((B, T, C), dtype=np.float32)
    for b in range(B):
        out[b] = np.asarray(res.results[2*b]["y"], dtype=np.float32)
        out[b] += np.asarray(res.results[2*b + 1]["y"], dtype=np.float32)
    return out
